# revision 1
# baseline (speedup 1.0000x reference)
"""LoRA-linear (dense fp32) on 8 Trainium2 NeuronCores.

out = x @ W_base.T + b_base + ((x @ A.T) @ B.T) * (alpha/r)

Full shapes: x [4, 2048, 4096] f32, W_base [4096, 4096], b_base [4096],
A [16, 4096], B [4096, 16]; out [4, 2048, 4096] f32.

Sharding: 4-way data-parallel over M = 4*2048 = 8192 flattened rows x
2-way tensor-parallel over out_features (4096 -> 2048 per group).
Core c handles m-rows [(c//2)*2048, ...) and out-cols [(c%2)*2048, ...).
A is replicated; b/B are sharded with out_features.

Per-core kernel (Tile framework):
  - All f32->bf16 casts ride on gpsimd casting DMAs (SWDGE can convert
    dtype in flight); no engine cycles are spent on conversion.
  - W shard: cast-DMA'd straight into SBUF row-blocks, transposed by the
    PE (bf16 transpose-mode) into the resident wt_sb[d, kt, o] (16MB).
    The PE transpose work (~512 tiles) overlaps the W DMA stream.
  - x shard: cast-DMA'd to a DRAM bf16 scratch, then XBAR DMA-transposed
    into [d, kt, m] tiles, one per 128-row m-tile, alternating the two
    HWDGE queues. The XBAR's ~35GB/s/queue is fine for x's 30GB/s
    demand, which is spread evenly across the kernel (W's is not: it is
    all needed up front, which is why W goes through the PE instead).
  - Each [128m, 512o] PSUM tile accumulates: 1 rank-1 matmul (ones x
    bias broadcast), 32 bf16 matmuls over d, and 1 K=16 LoRA matmul;
    evicted to f32 by DVE and DMA'd out.
  - LoRA: xa = x @ A.T per m-tile from the transposed x tiles; xa.T via
    one small PE transpose; scaling folded into B.T.
"""

import numpy as np

import concourse.bass as bass
import concourse.tile as tile
from concourse import bacc, mybir
from concourse import bass_utils
from concourse.bass import ts
from concourse.bass_interp import get_hw_module
from concourse.masks import make_identity

P = 128
D = 4096                 # in_features (contraction)
M_FULL = 8192            # 4 * 2048 flattened rows
O_FULL = 4096            # out_features
MGRID, OGRID = 4, 2      # core grid: 4 data-parallel x 2 tensor-parallel
M_SHARD = M_FULL // MGRID    # 2048
O_SHARD = O_FULL // OGRID    # 2048
KT = D // P              # 32 contraction tiles
MT = M_SHARD // P        # 16 m-tiles
OT = O_SHARD // P        # 16 o row-blocks of W shard
OC = 512                 # psum free dim per output tile
NOC = O_SHARD // OC      # 4
R = 16                   # lora rank
SCALING = 32.0 / 16.0    # alpha / r

F32 = mybir.dt.float32
BF16 = mybir.dt.bfloat16

_NC_CACHE = None


def _build_nc():
    nc = bacc.Bacc("TRN2", target_bir_lowering=False, debug=False, num_devices=8)
    x_d = nc.dram_tensor("x_s", [M_SHARD, D], F32, kind="ExternalInput").ap()
    w_d = nc.dram_tensor("w_s", [O_SHARD, D], F32, kind="ExternalInput").ap()
    b_d = nc.dram_tensor("b_s", [1, O_SHARD], F32, kind="ExternalInput").ap()
    a_d = nc.dram_tensor("a_r", [R, D], F32, kind="ExternalInput").ap()
    bm_d = nc.dram_tensor("bm_s", [O_SHARD, R], F32, kind="ExternalInput").ap()
    out_d = nc.dram_tensor("out_s", [M_SHARD, O_SHARD], F32, kind="ExternalOutput").ap()

    with tile.TileContext(nc) as tc:
        with (
            tc.tile_pool(name="const", bufs=1) as const,
            tc.tile_pool(name="wt", bufs=1) as wtp,
            tc.tile_pool(name="wrb", bufs=2) as wrbp,
            tc.tile_pool(name="xtp", bufs=3) as xtp,
            tc.tile_pool(name="ostage", bufs=3) as ostage,
            tc.tile_pool(name="small", bufs=2) as small,
            tc.tile_pool(name="dram_x", bufs=5, space="DRAM") as dram_x,
            tc.tile_pool(name="ps_out", bufs=4, space="PSUM") as ps_out,
            tc.tile_pool(name="ps_tp", bufs=2, space="PSUM") as ps_tp,
            tc.tile_pool(name="ps_sm", bufs=2, space="PSUM") as ps_sm,
        ):
            ident = const.tile([P, P], F32)
            make_identity(nc, ident)
            ident_bf = const.tile([P, P], BF16)
            make_identity(nc, ident_bf)
            ones = const.tile([1, P], BF16)
            nc.any.memset(ones[:], 1.0)

            # bias -> bf16 [1, O_SHARD] via casting DMA
            bias_sb = const.tile([1, O_SHARD], BF16)
            nc.gpsimd.dma_start(bias_sb[:], b_d[:, :])

            # A -> bf16 [128(pad), D] via casting DMA; PE-transpose to
            # at_sb[:, kt*R:(kt+1)*R] = A[:, kt*128:(kt+1)*128].T
            at_sb = const.tile([P, KT * R], BF16)
            a0 = const.tile([P, D], BF16)
            nc.any.memset(a0[:], 0.0)
            nc.gpsimd.dma_start(a0[0:R, :], a_d[:, :])
            for kt in range(KT):
                pst = ps_tp.tile([P, P], BF16, tag="tp")
                nc.tensor.transpose(pst[:], a0[:, ts(kt, P)], ident_bf[:])
                nc.vector.tensor_copy(at_sb[:, ts(kt, R)], pst[:, 0:R])

            # scaling * B.T -> bt_sb [R, O_SHARD] bf16
            bt_sb = const.tile([R, O_SHARD], BF16)
            bm3 = const.tile([P, OT, R], F32)
            nc.scalar.dma_start(bm3[:], bm_d.rearrange("(t p) r -> p t r", p=P))
            for t in range(OT):
                psb = ps_sm.tile([R, P], F32, tag="sm")
                nc.tensor.transpose(psb[:], bm3[:, t, :], ident[:])
                nc.scalar.mul(bt_sb[:, ts(t, P)], psb[:], SCALING)

            # Queue plan (measured best of 9 variants): gpsimd (SWDGE,
            # can cast in flight) alternates W and x cast-DMAs so both
            # streams ramp together; the sync HWDGE queue carries ONLY
            # XBAR xt transposes and scalar carries ONLY copy-mode
            # out-stores -- keeping each HWDGE queue in a single xbar
            # mode avoids the DMATranspose<->DMACopy transition hazard.
            wt_sb = wtp.tile([P, KT, O_SHARD], BF16)
            xt_tiles = [None] * MT

            def emit_x_stage(mi):
                xb = dram_x.tile([P, D], BF16, tag="xb", name=f"xb_{mi}")
                nc.gpsimd.dma_start(xb[:], x_d[ts(mi, P), :])
                xt = xtp.tile([P, KT, P], BF16, tag="xt", name=f"xt_{mi}")
                nc.sync.dma_start_transpose(xt[:, :, :], xb[:])
                xt_tiles[mi] = xt

            def emit_w_stage(wb):
                wrb = wrbp.tile([P, D], BF16, tag="wrb")
                nc.gpsimd.dma_start(wrb[:], w_d[ts(wb, P), :])
                for kt in range(KT):
                    pst = ps_tp.tile([P, P], BF16, tag="tp")
                    nc.tensor.transpose(pst[:], wrb[:, ts(kt, P)], ident_bf[:])
                    nc.vector.tensor_copy(wt_sb[:, kt, ts(wb, P)], pst[:])

            for wb in range(OT):
                emit_x_stage(wb)  # MT == OT: pair x m-tile wb with W block wb
                emit_w_stage(wb)

            # xa.T resident: [R, M_SHARD] bf16
            xat_sb = const.tile([R, M_SHARD], BF16)

            for mi in range(MT):
                xt = xt_tiles[mi]

                # xa[m, r] accumulation, then transpose to [r, m]
                psxa = ps_sm.tile([P, R], F32, tag="sm")
                for kt in range(KT):
                    nc.tensor.matmul(
                        psxa[:], xt[:, kt, :], at_sb[:, ts(kt, R)],
                        start=(kt == 0), stop=(kt == KT - 1),
                    )
                xa_sb = small.tile([P, R], F32, tag="xa")
                nc.vector.tensor_copy(xa_sb[:], psxa[:])
                psxat = ps_sm.tile([R, P], F32, tag="sm")
                nc.tensor.transpose(psxat[:], xa_sb[:], ident[:])
                nc.vector.tensor_copy(xat_sb[:, ts(mi, P)], psxat[:])

                # main accumulation groups: bias + 32 k-tiles + lora delta
                pso = [
                    ps_out.tile([P, OC], F32, tag="out", name=f"pso_{mi}_{i}")
                    for i in range(NOC)
                ]
                for oc in range(NOC):
                    nc.tensor.matmul(
                        pso[oc][:], ones[:], bias_sb[:, ts(oc, OC)],
                        start=True, stop=False,
                    )
                for kt in range(KT):
                    for oc in range(NOC):
                        nc.tensor.matmul(
                            pso[oc][:], xt[:, kt, :], wt_sb[:, kt, ts(oc, OC)],
                            start=False, stop=False,
                        )
                for oc in range(NOC):
                    nc.tensor.matmul(
                        pso[oc][:], xat_sb[:, ts(mi, P)], bt_sb[:, ts(oc, OC)],
                        start=False, stop=True,
                    )
                    ob = ostage.tile([P, OC], F32, tag="ob")
                    nc.vector.tensor_copy(ob[:], pso[oc][:])
                    nc.scalar.dma_start(out_d[ts(mi, P), ts(oc, OC)], ob[:])

    nc.compile()
    nc.m = get_hw_module(nc.m)
    return nc


def _get_nc():
    global _NC_CACHE
    if _NC_CACHE is None:
        _NC_CACHE = _build_nc()
    return _NC_CACHE


def _make_in_maps(x, W_base, b_base, A, B):
    xf = np.ascontiguousarray(np.asarray(x, np.float32).reshape(M_FULL, D))
    W = np.ascontiguousarray(np.asarray(W_base, np.float32))
    b = np.ascontiguousarray(np.asarray(b_base, np.float32))
    A = np.ascontiguousarray(np.asarray(A, np.float32))
    B = np.ascontiguousarray(np.asarray(B, np.float32))
    in_maps = []
    for c in range(MGRID * OGRID):
        i, j = divmod(c, OGRID)
        in_maps.append({
            "x_s": xf[i * M_SHARD:(i + 1) * M_SHARD],
            "w_s": np.ascontiguousarray(W[j * O_SHARD:(j + 1) * O_SHARD]),
            "b_s": np.ascontiguousarray(b[j * O_SHARD:(j + 1) * O_SHARD])[None, :],
            "a_r": A,
            "bm_s": np.ascontiguousarray(B[j * O_SHARD:(j + 1) * O_SHARD]),
        })
    return in_maps


def _gather(results):
    out = np.empty((M_FULL, O_FULL), np.float32)
    for c in range(MGRID * OGRID):
        i, j = divmod(c, OGRID)
        out[i * M_SHARD:(i + 1) * M_SHARD, j * O_SHARD:(j + 1) * O_SHARD] = \
            results[c]["out_s"]
    return out.reshape(4, 2048, 4096)


def run(x, W_base, b_base, A, B, trace=False, trace_kwargs=None):
    nc = _get_nc()
    in_maps = _make_in_maps(x, W_base, b_base, A, B)
    res = bass_utils.run_bass_kernel_spmd(
        nc, in_maps, core_ids=list(range(8)), trace=trace,
        **(trace_kwargs or {}),
    )
    return _gather(res.results), res


def kernel(x, W_base, b_base, A, B):
    out, _ = run(x, W_base, b_base, A, B, trace=False)
    return out



# revision 16
# speedup vs baseline: 1.0898x; 1.0898x over previous
"""LoRA-linear (dense fp32) on 8 Trainium2 NeuronCores.

out = x @ W_base.T + b_base + ((x @ A.T) @ B.T) * (alpha/r)

Full shapes: x [4, 2048, 4096] f32, W_base [4096, 4096], b_base [4096],
A [16, 4096], B [4096, 16]; out [4, 2048, 4096] f32.

Sharding: 4-way data-parallel over M = 4*2048 = 8192 flattened rows x
2-way tensor-parallel over out_features (4096 -> 2048 per group).
Core c handles m-rows [(c//2)*2048, ...) and out-cols [(c%2)*2048, ...).
A is replicated; b/B are sharded with out_features.

Per-core kernel (Tile framework), v2:
  - SWDGE (gpsimd) carries every cast: W row-blocks -> SBUF bf16
    (W-priority order), x m-tiles -> DRAM bf16 scratch, plus the f32
    out-stores.  The two HWDGE queues (sync/scalar) carry ONLY XBAR
    DMA-transposes of x (even/odd m-tiles split across them), so each
    queue stays in a single xbar mode and xt supply is ~2x the
    single-queue rate that throttled the previous version.
  - W blocks are PE-transposed (bf16) into the resident wt_sb.
  - Banded head: m-tiles 0-1 are emitted output-group by output-group,
    interleaved with the W band casts/transposes they depend on, so
    main matmuls start ~45us in instead of waiting for all of W.
  - xa.T ([r, m]) is computed directly per m-tile with the A k-tile as
    the stationary operand and xt as the 128-row moving operand; no
    separate xa transpose.
  - Bias is folded into the LoRA matmul: stationary [17, 128] =
    [xa.T; ones], moving [17, 512] = [scaling*B.T; bias], a single
    K=17 stop-matmul per psum group (bias start-matmul eliminated).
"""

import numpy as np

import concourse.bass as bass
import concourse.tile as tile
from concourse import bacc, mybir
from concourse import bass_utils
from concourse.bass import ts
from concourse.bass_interp import get_hw_module
from concourse.masks import make_identity

P = 128
D = 4096                 # in_features (contraction)
M_FULL = 8192            # 4 * 2048 flattened rows
O_FULL = 4096            # out_features
MGRID, OGRID = 4, 2      # core grid: 4 data-parallel x 2 tensor-parallel
M_SHARD = M_FULL // MGRID    # 2048
O_SHARD = O_FULL // OGRID    # 2048
KT = D // P              # 32 contraction tiles
MT = M_SHARD // P        # 16 m-tiles
OT = O_SHARD // P        # 16 o row-blocks of W shard
OC = 512                 # psum free dim per output tile
NOC = O_SHARD // OC      # 4
R = 16                   # lora rank
SCALING = 32.0 / 16.0    # alpha / r

F32 = mybir.dt.float32
BF16 = mybir.dt.bfloat16

_NC_CACHE = None


def _build_nc(hw=True):
    nc = bacc.Bacc("TRN2", target_bir_lowering=False, debug=False, num_devices=8)
    x_d = nc.dram_tensor("x_s", [M_SHARD, D], F32, kind="ExternalInput").ap()
    w_d = nc.dram_tensor("w_s", [O_SHARD, D], F32, kind="ExternalInput").ap()
    b_d = nc.dram_tensor("b_s", [1, O_SHARD], F32, kind="ExternalInput").ap()
    a_d = nc.dram_tensor("a_r", [R, D], F32, kind="ExternalInput").ap()
    bm_d = nc.dram_tensor("bm_s", [O_SHARD, R], F32, kind="ExternalInput").ap()
    out_d = nc.dram_tensor("out_s", [M_SHARD, O_SHARD], F32, kind="ExternalOutput").ap()

    with tile.TileContext(nc) as tc:
        with (
            tc.tile_pool(name="const", bufs=1) as const,
            tc.tile_pool(name="wt", bufs=1) as wtp,
            tc.tile_pool(name="wrb", bufs=2) as wrbp,
            tc.tile_pool(name="xtp", bufs=3) as xtp,
            tc.tile_pool(name="xbp", bufs=2) as xbp,
            tc.tile_pool(name="ostage", bufs=3) as ostage,
            tc.tile_pool(name="dram_x", bufs=1, space="DRAM") as dram_x,
            tc.tile_pool(name="ps_out", bufs=4, space="PSUM") as ps_out,
            tc.tile_pool(name="ps_tp", bufs=2, space="PSUM") as ps_tp,
            tc.tile_pool(name="ps_sm", bufs=2, space="PSUM") as ps_sm,
        ):
            ident = const.tile([P, P], F32)
            make_identity(nc, ident)
            ident_bf = const.tile([P, P], BF16)
            make_identity(nc, ident_bf)

            # A -> bf16 [R, D] via casting DMA; PE-transpose each k-tile
            # into at_sb[:, kt, :] = A[:, kt*128:(kt+1)*128].T
            a0 = const.tile([R, D], BF16)
            nc.gpsimd.dma_start(a0[:], a_d[:, :])

            # bt_aug: rows 0:16 = SCALING * B.T (bf16), row 16 = bias
            bt_aug = const.tile([R + 1, O_SHARD], BF16)
            nc.gpsimd.dma_start(bt_aug[R:R + 1, :], b_d[:, :])

            at_sb = const.tile([P, KT, R], BF16)
            for kt in range(KT):
                psat = ps_sm.tile([P, R], BF16, tag="sm")
                nc.tensor.transpose(
                    psat[:], a0[:, ts(kt, P)], ident_bf[0:R, 0:R])
                nc.vector.tensor_copy(at_sb[:, kt, :], psat[:])

            # Four rotating [17, 128] stationaries [xa.T; ones].  The ones
            # row (partition 16) can't be memset by a compute engine
            # (quarter-partition rule), so it is written once per buffer
            # by DMA via a DRAM bounce; per-iteration writes touch only
            # partitions 0:16.
            ones_sb = const.tile([1, P], BF16)
            nc.any.memset(ones_sb[:], 1.0)
            ones_dram = dram_x.tile([1, P], BF16, tag="ones")
            nc.gpsimd.dma_start(ones_dram[:], ones_sb[:])
            xat_bufs = []
            for i in range(4):
                xb_ = const.tile([R + 1, P], BF16, name=f"xatbuf{i}")
                nc.gpsimd.dma_start(xb_[R:R + 1, :], ones_dram[:])
                xat_bufs.append(xb_)

            bm3 = const.tile([P, OT, R], F32)
            nc.scalar.dma_start(bm3[:], bm_d.rearrange("(t p) r -> p t r", p=P))
            for t in range(OT):
                psb = ps_sm.tile([R, P], F32, tag="sm")
                nc.tensor.transpose(psb[:], bm3[:, t, :], ident[:])
                nc.scalar.mul(bt_aug[0:R, ts(t, P)], psb[:], SCALING)

            # Resident transposed W: wt_sb[d, kt, o]
            wt_sb = wtp.tile([P, KT, O_SHARD], BF16)
            xt_tiles = [None] * MT
            xat_tiles = [None] * MT

            def emit_w(wb):
                """Cast W row-block wb (SWDGE) and PE-transpose it into
                wt_sb columns [wb*128, (wb+1)*128)."""
                wrb = wrbp.tile([P, D], BF16, tag="wrb")
                nc.gpsimd.dma_start(wrb[:], w_d[ts(wb, P), :])
                for kt in range(KT):
                    pst = ps_tp.tile([P, P], BF16, tag="tp")
                    nc.tensor.transpose(pst[:], wrb[:, ts(kt, P)], ident_bf[:])
                    nc.vector.tensor_copy(wt_sb[:, kt, ts(wb, P)], pst[:])

            def emit_xcast(mi):
                """x m-tile -> SBUF bf16 staging (SWDGE cast)."""
                xb = xbp.tile([P, D], BF16, tag="xb", name=f"xb_{mi}")
                nc.gpsimd.dma_start(xb[:], x_d[ts(mi, P), :])
                xt_tiles[mi] = ("staged", xb)

            def emit_xtp(mi):
                """PE-transpose the staged x m-tile into xt[d, kt, m] --
                same proven cast->SBUF->PE->DVE pattern as the W path
                (the DRAM-scratch XBAR route raced nondeterministically
                on hardware)."""
                _, xb = xt_tiles[mi]
                xt = xtp.tile([P, KT, P], BF16, tag="xt", name=f"xt_{mi}")
                for kt in range(KT):
                    pst = ps_tp.tile([P, P], BF16, tag="tp")
                    nc.tensor.transpose(pst[:], xb[:, ts(kt, P)], ident_bf[:])
                    nc.vector.tensor_copy(xt[:, kt, :], pst[:])
                xt_tiles[mi] = xt

            def emit_xa(mi):
                """xa.T for m-tile mi: [R, 128] psum accumulated over kt
                with the A k-tile stationary; then build the [17, 128]
                stationary [xa.T; ones] in SBUF bf16."""
                xt = xt_tiles[mi]
                psxat = ps_sm.tile([R, P], F32, tag="sm")
                for kt in range(KT):
                    nc.tensor.matmul(
                        psxat[:], at_sb[:, kt, :], xt[:, kt, :],
                        start=(kt == 0), stop=(kt == KT - 1),
                    )
                xat = xat_bufs[mi % 4]
                nc.vector.tensor_copy(xat[0:R, :], psxat[:])
                xat_tiles[mi] = xat

            def emit_g(mi, g):
                """One [128m, 512o] psum group: 32 k-tile matmuls plus the
                merged lora+bias K=17 stop-matmul; evict and store."""
                xt = xt_tiles[mi]
                pso = ps_out.tile([P, OC], F32, tag="out", name=f"pso_{mi}_{g}")
                for kt in range(KT):
                    nc.tensor.matmul(
                        pso[:], xt[:, kt, :], wt_sb[:, kt, ts(g, OC)],
                        start=(kt == 0), stop=False,
                    )
                nc.tensor.matmul(
                    pso[:], xat_tiles[mi][:], bt_aug[:, ts(g, OC)],
                    start=False, stop=True,
                )
                ob = ostage.tile([P, OC], F32, tag="ob")
                nc.vector.tensor_copy(ob[:], pso[:])
                nc.scalar.dma_start(out_d[ts(mi, P), ts(g, OC)], ob[:])

            # ---- banded head: W bands interleaved with m-tiles 0-1.
            # Group (mi, g) reads W blocks 4g..4g+3, so every emit_g(_, g)
            # must come after emit_w(4g+3).  SWDGE order puts x0 first,
            # then W blocks with the next few x casts woven in.
            # xb staging has bufs=2, so emit_xcast(k) must come after
            # emit_xtp(k-2) (the prior slot tenant's readers must be
            # registered before the slot is re-allocated).
            emit_xcast(0)
            emit_w(0); emit_w(1)
            emit_xcast(1)
            emit_w(2); emit_w(3)
            emit_xtp(0)
            emit_xa(0); emit_g(0, 0)
            emit_w(4)
            emit_xtp(1)
            emit_xcast(2)
            emit_xa(1); emit_g(1, 0)
            emit_w(5); emit_w(6); emit_w(7)
            emit_g(0, 1); emit_g(1, 1)
            emit_xcast(3)
            emit_w(8); emit_w(9); emit_w(10); emit_w(11)
            emit_xtp(2)
            emit_xcast(4)
            emit_g(0, 2); emit_g(1, 2)
            emit_w(12); emit_w(13); emit_w(14); emit_w(15)
            emit_g(0, 3); emit_g(1, 3)

            # ---- steady state: transpose the next tile between the
            # current tile's first and second psum groups ----
            for mi in range(2, MT):
                emit_xa(mi)
                emit_g(mi, 0)
                if mi + 1 <= MT - 1:
                    emit_xtp(mi + 1)
                if mi + 3 <= MT - 1:
                    emit_xcast(mi + 3)
                for g in range(1, NOC):
                    emit_g(mi, g)

    nc.compile()
    if hw:
        nc.m = get_hw_module(nc.m)
    return nc


def _get_nc():
    global _NC_CACHE
    if _NC_CACHE is None:
        _NC_CACHE = _build_nc()
    return _NC_CACHE


def _make_in_maps(x, W_base, b_base, A, B):
    xf = np.ascontiguousarray(np.asarray(x, np.float32).reshape(M_FULL, D))
    W = np.ascontiguousarray(np.asarray(W_base, np.float32))
    b = np.ascontiguousarray(np.asarray(b_base, np.float32))
    A = np.ascontiguousarray(np.asarray(A, np.float32))
    B = np.ascontiguousarray(np.asarray(B, np.float32))
    in_maps = []
    for c in range(MGRID * OGRID):
        i, j = divmod(c, OGRID)
        in_maps.append({
            "x_s": xf[i * M_SHARD:(i + 1) * M_SHARD],
            "w_s": np.ascontiguousarray(W[j * O_SHARD:(j + 1) * O_SHARD]),
            "b_s": np.ascontiguousarray(b[j * O_SHARD:(j + 1) * O_SHARD])[None, :],
            "a_r": A,
            "bm_s": np.ascontiguousarray(B[j * O_SHARD:(j + 1) * O_SHARD]),
        })
    return in_maps


def _gather(results):
    out = np.empty((M_FULL, O_FULL), np.float32)
    for c in range(MGRID * OGRID):
        i, j = divmod(c, OGRID)
        out[i * M_SHARD:(i + 1) * M_SHARD, j * O_SHARD:(j + 1) * O_SHARD] = \
            results[c]["out_s"]
    return out.reshape(4, 2048, 4096)


def run(x, W_base, b_base, A, B, trace=False, trace_kwargs=None):
    nc = _get_nc()
    in_maps = _make_in_maps(x, W_base, b_base, A, B)
    res = bass_utils.run_bass_kernel_spmd(
        nc, in_maps, core_ids=list(range(8)), trace=trace,
        **(trace_kwargs or {}),
    )
    return _gather(res.results), res


def kernel(x, W_base, b_base, A, B):
    out, _ = run(x, W_base, b_base, A, B, trace=False)
    return out
